# revision 2
# baseline (speedup 1.0000x reference)
"""GCN encoder (concat-edges GCNConv) as a distributed Bass/Tile kernel on
8 NeuronCores — v3: residue-split SWDGE gather from a compact fp16 table.

Strategy: nodes/output sharded 8 ways; edges partitioned by destination owner;
per-core replicated fp16 feature table via compact AllGather (6.4MB). Gather
calls are split by r = src %% 4 and read one of four 64B-offset views of the
table whose 256B SWDGE elements start exactly at src's row (idx = src // 4),
so no 256B-row padding or relayout is needed. Slots are tight-packed per
(dst-section, residue) with SPMD-max padding only (~2.6%%); aggregation is
mask-matmuls driven by a union job list so all cores run one instruction
stream; the finalize folds +h'+b into an identity matmul per group and the
deg^-1/2 scale into the PSUM->SBUF copy.

Math: out = dinv * (S @ (dinv * (x@W)) + dinv*(x@W)) + b, S the real-edge
adjacency; self-loops analytic; dinv = rsqrt(deg+1); deg from host prefix
offsets (layout metadata). Host does layout only: sorting, bucketing, slot
packing, job lists, prefix offsets.
"""
import sys

if "/opt/trn_rl_repo" not in sys.path:
    sys.path.insert(0, "/opt/trn_rl_repo")

import numpy as np

P = 128
LAT = 32
IN = 128
N = 100_000
NC = 8
SH = 12_544
NPAD = NC * SH          # 100352 = 4 * 25088
NR = 4                  # src residues
KROWS = NPAD // NR      # 25088 view rows, int16-safe
SEC_G = [13, 13, 12, 12, 12, 12, 12, 12]   # groups per section (sum 98)
NG = SH // P            # 98
GCH = 7                 # tiles per gather call


def prepare(x, edge_index, y_edge_index, W, b):
    assert sum(SEC_G) == NG
    g2s = np.repeat(np.arange(len(SEC_G)), SEC_G)

    ei = np.concatenate([np.asarray(edge_index), np.asarray(y_edge_index)],
                        axis=1)
    src_g = ei[0].astype(np.int64)
    dst_g = ei[1].astype(np.int64)
    owner = dst_g // SH

    per_core = []
    cnt = np.zeros((NC, len(SEC_G), NR), np.int64)
    for c in range(NC):
        sel = owner == c
        s = src_g[sel].astype(np.int32)
        d = (dst_g[sel] - c * SH).astype(np.int32)
        r = s % NR
        sec = g2s[d // P]
        order = np.lexsort((d, r, sec))
        s, d, r, sec = s[order], d[order], r[order], sec[order]
        np.add.at(cnt[c], (sec, r), 1)
        per_core.append((s, d, r, sec))

    cap = cnt.max(axis=0)                      # [NSEC, NR]
    T_sr = -(-cap // P)
    T = int(T_sr.sum())
    tile_base = np.zeros((len(SEC_G), NR), np.int64)
    run = 0
    for si in range(len(SEC_G)):
        for ri in range(NR):
            tile_base[si, ri] = run
            run += T_sr[si, ri]
    nslots = T * P

    BIGD = np.int32(1 << 20)
    idx16 = np.zeros((NC, nslots), np.int16)
    dslot = np.full((NC, nslots), BIGD, np.int32)
    for c in range(NC):
        s, d, r, sec = per_core[c]
        pos = np.zeros(len(d), np.int64)
        for si in range(len(SEC_G)):
            for ri in range(NR):
                m = (sec == si) & (r == ri)
                k = int(m.sum())
                pos[m] = tile_base[si, ri] * P + np.arange(k)
        idx16[c, pos] = (s // NR).astype(np.int16)
        dslot[c, pos] = d

    # union job list, ordered (sec, g, r, t)
    tile_groups = []
    for t in range(T):
        dv = dslot[:, t * P:(t + 1) * P]
        dv = dv[dv < BIGD]
        gs = sorted(set((dv // P).tolist())) if len(dv) else []
        tile_groups.append(gs)
    tg_by_g = {}
    for t, gs in enumerate(tile_groups):
        for g in gs:
            tg_by_g.setdefault(g, []).append(t)
    assert len(tg_by_g) == NG, "every dst-group needs at least one job"
    job_t, job_g = [], []
    for g in range(NG):
        for t in tg_by_g[g]:
            job_t.append(t)
            job_g.append(g)
    J = len(job_t)
    Jpad = -(-J // 8) * 8

    dr2 = np.full((NC, Jpad, P), 4096.0, np.float32)
    for j, (t, g) in enumerate(zip(job_t, job_g)):
        dv = dslot[:, t * P:(t + 1) * P].astype(np.float64) - g * P
        dv[dv > 2048] = 4096.0
        dr2[:, j, :] = dv
    first = np.zeros(J, bool)
    last = np.zeros(J, bool)
    seen = set()
    for j in range(J):
        if job_g[j] not in seen:
            first[j] = True
            seen.add(job_g[j])
    seen = set()
    for j in range(J - 1, -1, -1):
        if job_g[j] not in seen:
            last[j] = True
            seen.add(job_g[j])

    calls = []        # (r, t0, t1)
    chunk_of = {}
    for si in range(len(SEC_G)):
        for ri in range(NR):
            t0 = int(tile_base[si, ri])
            t1 = int(t0 + T_sr[si, ri])
            t = t0
            while t < t1:
                te = min(t + GCH, t1)
                for tt in range(t, te):
                    chunk_of[tt] = len(calls)
                calls.append((ri, t, te))
                t = te

    idxw = np.zeros((NC, P, T * 8), np.int16)
    for c in range(NC):
        w = idx16[c].reshape(T * 8, 16).T
        idxw[c] = np.tile(w, (8, 1))

    iota128 = np.tile(np.arange(P, dtype=np.float32), (P, 1)).astype(
        np.float16)
    ident = np.eye(P, dtype=np.float16)
    b_rep = np.tile(np.asarray(b, np.float32)[None, :], (P, 1))
    W32 = np.asarray(W, np.float32)

    x = np.asarray(x, np.float32)
    xpad = np.zeros((NPAD, IN), np.float32)
    xpad[:N] = x

    in_maps = []
    for c in range(NC):
        _, d_all, _, _ = per_core[c]
        deg_off = np.concatenate(
            [[0], np.cumsum(np.bincount(d_all, minlength=SH))]).astype(
                np.float32)
        xt = np.ascontiguousarray(xpad[c * SH:(c + 1) * SH].T)
        in_maps.append({
            "xT": xt,
            "W": W32,
            "b_rep": b_rep,
            "iota128": iota128,
            "ident": ident,
            "idxs": np.ascontiguousarray(idxw[c]),
            "dr2": np.ascontiguousarray(
                dr2[c].transpose(1, 0).astype(np.float16)),
            "slo": np.ascontiguousarray(deg_off[:SH].reshape(NG, P).T),
            "shi": np.ascontiguousarray(deg_off[1:SH + 1].reshape(NG, P).T),
        })

    meta = {
        "T": T, "J": J, "Jpad": Jpad,
        "job_t": job_t, "job_g": job_g,
        "first": first.tolist(), "last": last.tolist(),
        "calls": calls, "chunk_of": chunk_of,
        "nslots": nslots,
    }
    return in_maps, meta


def host_simulate(x, edge_index, y_edge_index, W, b, in_maps, meta):
    x = np.asarray(x, np.float32)
    W32 = np.asarray(W, np.float32)
    xpad = np.zeros((NPAD, IN), np.float32)
    xpad[:N] = x

    ei = np.concatenate([np.asarray(edge_index), np.asarray(y_edge_index)],
                        axis=1)
    deg_full = np.zeros(NPAD, np.float32)
    np.add.at(deg_full, ei[1].astype(np.int64), 1.0)
    dinv_full = 1.0 / np.sqrt(deg_full + 1.0)
    h_full = (xpad @ W32) * dinv_full[:, None]
    h16 = h_full.astype(np.float16)

    T = meta["T"]
    rof = np.zeros(T, np.int64)
    for (ri, t0, t1) in meta["calls"]:
        rof[t0:t1] = ri

    outs = []
    for c in range(NC):
        m = in_maps[c]
        deg = m["shi"] - m["slo"]
        dinv = 1.0 / np.sqrt(deg + 1.0)
        idxw = m["idxs"]
        idx = np.zeros(T * P, np.int64)
        for t in range(T):
            w = idxw[:16, t * 8:(t + 1) * 8]
            idx[t * P:(t + 1) * P] = w.T.reshape(P)
        rows = idx * NR + rof.repeat(P)
        gath = h16[rows].astype(np.float32)

        acc = np.zeros((SH, LAT), np.float32)
        dr2 = m["dr2"].astype(np.float32)
        for j in range(meta["J"]):
            t = meta["job_t"][j]
            g = meta["job_g"][j]
            col = dr2[:, j]
            ok = (col >= 0) & (col < P)
            mask = np.zeros((P, P), np.float32)
            mask[np.where(ok)[0], col[ok].astype(np.int64)] = 1.0
            acc[g * P:(g + 1) * P] += mask.T @ gath[t * P:(t + 1) * P]

        h_own = h_full[c * SH:(c + 1) * SH]
        dinv_flat = np.array([dinv[d % P, d // P] for d in range(SH)],
                             np.float32)
        out = dinv_flat[:, None] * (acc + h_own) + np.asarray(b, np.float32)
        outs.append(out)
    return np.concatenate(outs, axis=0)[:N]


G_BUFS = 26
MASK_BUFS = 10
MC = 8


def build_module(meta):
    import concourse.bacc as bacc
    import concourse.tile as tile
    import concourse.mybir as mybir

    T, J, Jpad = meta["T"], meta["J"], meta["Jpad"]
    job_t, job_g = meta["job_t"], meta["job_g"]
    first, last = meta["first"], meta["last"]
    calls, chunk_of = meta["calls"], meta["chunk_of"]

    nc = bacc.Bacc("TRN2", target_bir_lowering=False, debug=False,
                   enable_asserts=False, num_devices=NC,
                   num_swdge_queues=4)

    dt = mybir.dt
    xT_d = nc.dram_tensor("xT", [IN, SH], dt.float32, kind="ExternalInput")
    W_d = nc.dram_tensor("W", [IN, LAT], dt.float32, kind="ExternalInput")
    brep_d = nc.dram_tensor("b_rep", [P, LAT], dt.float32, kind="ExternalInput")
    iota_d = nc.dram_tensor("iota128", [P, P], dt.float16, kind="ExternalInput")
    ident_d = nc.dram_tensor("ident", [P, P], dt.float16, kind="ExternalInput")
    idxs_d = nc.dram_tensor("idxs", [P, T * 8], dt.int16, kind="ExternalInput")
    dr2_d = nc.dram_tensor("dr2", [P, Jpad], dt.float16, kind="ExternalInput")
    slo_d = nc.dram_tensor("slo", [P, NG], dt.float32, kind="ExternalInput")
    shi_d = nc.dram_tensor("shi", [P, NG], dt.float32, kind="ExternalInput")
    out_d = nc.dram_tensor("out", [SH, LAT], dt.float32, kind="ExternalOutput")

    AF = mybir.ActivationFunctionType
    OP = mybir.AluOpType

    with tile.TileContext(nc) as tc:
        with tc.tile_pool(name="res", bufs=1) as res, \
             tc.tile_pool(name="dram", bufs=1, space="DRAM") as dram:
            idxs_t = res.tile([P, T * 8], dt.int16)
            dr2_t = res.tile([P, Jpad], dt.float16)
            iota_t = res.tile([P, P], dt.float16)
            ident_t = res.tile([P, P], dt.float16)
            W_t = res.tile([IN, LAT], dt.float32)
            Wb_t = res.tile([IN, LAT], dt.bfloat16)
            brep_t = res.tile([P, LAT], dt.float32)
            slo_t = res.tile([P, NG], dt.float32)
            shi_t = res.tile([P, NG], dt.float32)
            dinv_t = res.tile([P, NG], dt.float32)
            h128 = res.tile([P, NG * LAT], dt.float32)
            h16 = res.tile([P, NG * LAT], dt.float16)
            acc128 = res.tile([P, NG * LAT], dt.float32)
            warm = res.tile([P, 512], dt.float32)
            gwarm = res.tile([P, P], dt.float16)

            h_shard = dram.tile([SH, LAT], dt.float16)
            h_all = dram.tile([NPAD + NR, LAT], dt.float16,
                              addr_space="Shared")
            dum_d = dram.tile([NR, LAT], dt.float16)

            nc.sync.dma_start(idxs_t[:], idxs_d[:])
            nc.sync.dma_start(dr2_t[:], dr2_d[:])
            nc.sync.dma_start(iota_t[:], iota_d[:])
            nc.sync.dma_start(ident_t[:], ident_d[:])
            nc.sync.dma_start(W_t[:], W_d[:])
            nc.sync.dma_start(brep_t[:], brep_d[:])
            nc.sync.dma_start(slo_t[:], slo_d[:])
            nc.sync.dma_start(shi_t[:], shi_d[:])

            # warm the SWDGE gather library during phase A (IRAM load)
            zidx = res.tile([P, 8], dt.int16)
            nc.vector.memset(zidx[:], 0)
            nc.sync.dma_start(dum_d[:], iota_d[:NR, :LAT])
            nc.gpsimd.dma_gather(
                gwarm[:].rearrange("p (o n) -> p o n", o=1),
                dum_d[:].rearrange("(k four) f -> k (four f)", four=NR),
                zidx[:], P, P, P, queue_num=0)

            nc.vector.tensor_tensor(out=slo_t[:], in0=shi_t[:], in1=slo_t[:],
                                    op=OP.subtract)
            nc.scalar.activation(shi_t[:], slo_t[:], AF.Sqrt, bias=1.0)
            nc.vector.reciprocal(dinv_t[:], shi_t[:])
            nc.scalar.activation(Wb_t[:], W_t[:], AF.Copy)

            # ---------------- phase A: h' = dinv * (x @ W) ------------------
            with tc.tile_pool(name="xt", bufs=1) as xtp, \
                 tc.tile_pool(name="psA", bufs=2, space="PSUM") as psA:
                nc.vector.memset(warm[:], 1.0)
                pw = psA.tile([P, 512], dt.float32, tag="h")
                for _ in range(10):
                    nc.tensor.matmul(out=pw[:], lhsT=warm[:, :P],
                                     rhs=warm[:], start=True, stop=True)
                nc.scalar.activation(warm[:, :1], pw[:, :1], AF.Copy)

                xb_t = xtp.tile([IN, SH], dt.bfloat16)
                XCH = SH // 4
                for k in range(4):  # chunked fp32->bf16 cast load (gpsimd)
                    nc.gpsimd.dma_start(out=xb_t[:, k * XCH:(k + 1) * XCH],
                                        in_=xT_d[:, k * XCH:(k + 1) * XCH])
                for g in range(NG):
                    ph = psA.tile([P, LAT], dt.float32, tag="h")
                    nc.tensor.matmul(out=ph[:], lhsT=xb_t[:, g * P:(g + 1) * P],
                                     rhs=Wb_t[:], start=True, stop=True)
                    nc.vector.tensor_tensor(
                        out=h128[:, g * LAT:(g + 1) * LAT],
                        in0=ph[:],
                        in1=dinv_t[:, g:g + 1].to_broadcast([P, LAT]),
                        op=OP.mult)

            nc.scalar.activation(h16[:], h128[:], AF.Copy)
            nc.sync.dma_start(
                h_shard.rearrange("(g p) f -> p g f", p=P),
                h16[:].rearrange("p (g f) -> p g f", f=LAT))
            nc.gpsimd.collective_compute(
                "AllGather", OP.bypass,
                replica_groups=[list(range(NC))],
                ins=[h_shard[:]], outs=[h_all[:NPAD, :]])

            # hs2 = h' + b/dinv  (fp16; final out = dinv*(acc + hs2))
            sdinv = res.tile([P, NG], dt.float32)
            hs2 = res.tile([P, NG * LAT], dt.float16)
            nc.vector.reciprocal(sdinv[:], dinv_t[:])
            nc.vector.tensor_tensor(
                out=hs2[:].rearrange("p (g f) -> p g f", f=LAT),
                in0=sdinv[:, :, None].to_broadcast([P, NG, LAT]),
                in1=brep_t[:, None, :].to_broadcast([P, NG, LAT]),
                op=OP.mult)
            nc.vector.tensor_tensor(out=hs2[:], in0=hs2[:], in1=h128[:],
                                    op=OP.add)

            # ------- pass 2: gather calls + mask-matmul job sweep ----------
            views = [
                h_all[ri:ri + NPAD, :].rearrange("(k four) f -> k (four f)",
                                                 four=NR)
                for ri in range(NR)
            ]
            with tc.tile_pool(name="gat", bufs=G_BUFS) as gp, \
                 tc.tile_pool(name="mask2", bufs=MASK_BUFS) as mp2, \
                 tc.tile_pool(name="psG", bufs=8, space="PSUM") as psG:
                gtiles = {}
                masks = {}

                def get_chunk(ci):
                    if ci not in gtiles:
                        ri, t0, t1 = calls[ci]
                        cw = t1 - t0
                        gt = gp.tile([P, GCH * P], dt.float16, tag="g")
                        nc.gpsimd.dma_gather(
                            gt[:, :cw * P].rearrange("p (t e) -> p t e", e=P),
                            views[ri],
                            idxs_t[:, t0 * 8:t1 * 8],
                            cw * P, cw * P, P,
                            queue_num=ci % 4)
                        gtiles[ci] = gt
                    return gtiles[ci]

                def get_mask(mj):
                    if mj not in masks:
                        cw = min(MC, Jpad - mj * MC)
                        mt = mp2.tile([P, MC * P], dt.float16, tag="m2")
                        nc.vector.tensor_tensor(
                            out=mt[:, :cw * P]
                                .rearrange("p (t f) -> p t f", t=cw),
                            in0=dr2_t[:, mj * MC:mj * MC + cw, None]
                                .to_broadcast([P, cw, P]),
                            in1=iota_t[:, None, :].to_broadcast([P, cw, P]),
                            op=OP.is_equal)
                        masks[mj] = mt
                    return masks[mj]

                pa = None
                cur_g = None
                for j in range(J):
                    t, g = job_t[j], job_g[j]
                    ci = chunk_of[t]
                    _, ct0, _ = calls[ci]
                    mj, mo = divmod(j, MC)
                    if first[j]:
                        pa = psG.tile([P, LAT], dt.float32, tag="agg")
                        cur_g = g
                    assert cur_g == g
                    nc.tensor.matmul(
                        out=pa[:],
                        lhsT=get_mask(mj)[:, mo * P:(mo + 1) * P],
                        rhs=get_chunk(ci)[:, (t - ct0) * P:(t - ct0) * P + LAT],
                        start=bool(first[j]), stop=False)
                    if last[j]:
                        # psum += hs2 (identity matmul), closing the group
                        nc.tensor.matmul(
                            out=pa[:], lhsT=ident_t[:],
                            rhs=hs2[:, g * LAT:(g + 1) * LAT],
                            start=False, stop=True)
                        # out = dinv * psum  (scaled copy), then store
                        a = acc128[:, g * LAT:(g + 1) * LAT]
                        nc.scalar.activation(a, pa[:], AF.Copy,
                                             scale=dinv_t[:, g:g + 1])
                        nc.sync.dma_start(
                            out_d[g * P:(g + 1) * P, :], a)

    nc.compile()
    return nc


LAST_EXEC_NS = None


def kernel(x, edge_index, y_edge_index, W, b):
    import os
    global LAST_EXEC_NS
    from concourse import bass_utils

    in_maps, meta = prepare(x, edge_index, y_edge_index, W, b)
    nc = build_module(meta)
    trace = os.environ.get("KERNEL_TRACE", "0") == "1"
    res = bass_utils.run_bass_kernel_spmd(nc, in_maps,
                                          core_ids=list(range(NC)),
                                          trace=trace)
    if trace:
        LAST_EXEC_NS = res.exec_time_ns
        print("exec_time_ns:", res.exec_time_ns, flush=True)
    outs = [res.results[c]["out"] for c in range(NC)]
    return np.concatenate(outs, axis=0)[:N].astype(np.float32)
